# revision 37
# baseline (speedup 1.0000x reference)
"""Trainium2 Bass kernel for a 2-layer SuperGAT (MX mode) GNN.

Strategy (graph/data parallel across 8 NeuronCores):
  - Nodes are sharded contiguously across cores by destination id (the
    segment-softmax key), so all attention normalization is core-local.
  - Each core sorts its nodes by in-degree and processes them in tiles of
    128 nodes x D slots (D = per-tile max degree, uniform across cores),
    so segment-softmax/aggregation become dense strided tree reductions
    with ~1% slot padding. Padded slots point at a dummy table row whose
    alpha_l = -60000 forces their softmax weight to exp(-inf) = 0.
  - Per-layer node records [h | alpha_l | alpha_r] (fp16) are written to
    a local HBM table and AllGather'd across the 8 cores; per-edge source
    records are fetched with per-slot-column indirect DMA ([128,1] row
    offsets - the only indexed-DMA shape this stack lowers correctly).
  - All floating-point math runs on-chip: TensorE for x@W1 / x1@W2 (the
    elu(v)+1 trick folds the "-1" into an extra all-ones contraction row),
    DVE for per-edge products and fp16 tree reductions, ScalarE for
    sigmoid/exp/log-softmax pieces. Host work is index bookkeeping only.
"""

import sys

import numpy as np


def _import_concourse():
    try:
        import concourse.bass  # noqa: F401
    except ImportError:
        for p in ("/opt/trn_rl_repo", "/root/.axon_site/_ro/trn_rl_repo"):
            if p not in sys.path:
                sys.path.insert(0, p)
        import concourse.bass  # noqa: F401


# ----------------------------------------------------------------------------
# Configuration
# ----------------------------------------------------------------------------

FULL_CFG = dict(
    N=100_000, E=1_600_000, F_IN=512, H=8, C1=8, NCLS=16,
    NCORES=8, NEG_SLOPE=0.2,
)

DUMMY_AL = -60000.0  # fp16-representable; kills softmax weight of pad slots


# ----------------------------------------------------------------------------
# Host-side graph preparation
# ----------------------------------------------------------------------------

def _prepare(cfg, x, edge_index):
    """Shard + degree-sort + slot layout (int32 row indices per slot)."""
    N, F = cfg["N"], cfg["F_IN"]
    NC = cfg["NCORES"]
    NSH = N // NC
    assert NSH * NC == N

    src = np.asarray(edge_index[0], dtype=np.int64)
    dst = np.asarray(edge_index[1], dtype=np.int64)
    loop = np.arange(N, dtype=np.int64)
    src = np.concatenate([src, loop])
    dst = np.concatenate([dst, loop])

    deg = np.bincount(dst, minlength=N)  # >= 1 for all nodes (self loop)

    NT = (NSH + 127) // 128
    NPC = NT * 128
    LT = NPC + 128                   # +1 dummy tile of table rows
    NFAKE = NPC - NSH

    pos_of = np.full(N, -1, dtype=np.int64)
    nodes_at = np.zeros((NC, NPC), dtype=np.int64)
    tile_maxdeg = np.zeros((NC, NT), dtype=np.int64)
    for c in range(NC):
        ids = np.arange(c * NSH, (c + 1) * NSH)
        order = np.argsort(deg[ids], kind="stable")
        ordered = ids[order]
        pos_of[ordered] = NFAKE + np.arange(NSH)
        na = np.full(NPC, -1, dtype=np.int64)
        na[NFAKE:] = ordered
        nodes_at[c] = na
        d = np.ones(NPC, dtype=np.int64)
        d[NFAKE:] = deg[ordered]
        tile_maxdeg[c] = d.reshape(NT, 128).max(axis=1)

    plan = np.maximum(tile_maxdeg.max(axis=0), 1).astype(np.int64)
    tile_ofs = np.concatenate([[0], np.cumsum(128 * plan)])
    GTOT = int(tile_ofs[-1])

    row_of = np.zeros(N, dtype=np.int64)
    for c in range(NC):
        ids = np.arange(c * NSH, (c + 1) * NSH)
        row_of[ids] = c * LT + pos_of[ids]

    gidx = np.zeros((NC, GTOT), dtype=np.int32)
    KCH = F // 128
    assert KCH * 128 == F
    xt = np.zeros((NC, NT, KCH, 128, 128), dtype=np.float32)
    x = np.asarray(x, np.float32)
    shard_of = dst // NSH
    for c in range(NC):
        dummy_row = c * LT + NPC
        g = np.full(GTOT, dummy_row, dtype=np.int64)
        es = np.nonzero(shard_of == c)[0]
        p = pos_of[dst[es]]
        o = np.argsort(p, kind="stable")
        p_s = p[o]
        src_s = src[es][o]
        starts = np.searchsorted(p_s, np.arange(NPC))
        rank = np.arange(len(p_s)) - starts[p_s]
        t = p_s >> 7
        # layout per tile: [128, D] row-major: addr = ofs + p*D + d
        addr = tile_ofs[t] + (p_s & 127) * plan[t] + rank
        g[addr] = row_of[src_s]
        # fake nodes: slot 0 -> own (zero) row so den = 1
        for tt in range(NT):
            fk = np.nonzero(nodes_at[c][tt * 128:(tt + 1) * 128] < 0)[0]
            if len(fk):
                g[tile_ofs[tt] + fk * plan[tt]] = c * LT + tt * 128 + fk
        gidx[c] = g.astype(np.int32)

        xs = np.zeros((NPC, F), dtype=np.float32)
        real = nodes_at[c] >= 0
        xs[real] = x[nodes_at[c][real]]
        xt[c] = xs.reshape(NT, 128, KCH, 128).transpose(0, 2, 3, 1).copy()

    info = dict(plan=plan, tile_ofs=tile_ofs, GTOT=GTOT, NT=NT, NPC=NPC,
                LT=LT, NSH=NSH, pos_of=pos_of, nodes_at=nodes_at)
    return gidx, xt, info


def _const_inputs(cfg, W1, att_l1, att_r1, bias1, W2, att_l2, att_r2, bias2):
    H, C1, NCLS, F = cfg["H"], cfg["C1"], cfg["NCLS"], cfg["F_IN"]
    HC1, HC2 = H * C1, H * NCLS
    REC1, REC2 = HC1 + 2 * H, HC2 + 2 * H
    KCH = F // 128

    w1 = np.ascontiguousarray(
        np.asarray(W1, np.float32).reshape(KCH, 128, HC1))
    W2 = np.asarray(W2, np.float32)
    w2e = np.ascontiguousarray(
        np.concatenate([W2, -W2.sum(axis=0, keepdims=True)],
                       axis=0).astype(np.float32))

    def rep(a):
        return np.ascontiguousarray(
            np.tile(np.asarray(a, np.float32).reshape(1, -1), (128, 1)))

    d1 = np.zeros((128, REC1), np.float16)
    d1[:, HC1:HC1 + H] = DUMMY_AL
    d2 = np.zeros((128, REC2), np.float16)
    d2[:, HC2:HC2 + H] = DUMMY_AL
    return dict(w1=w1, w2e=w2e, al1r=rep(att_l1), ar1r=rep(att_r1),
                al2r=rep(att_l2), ar2r=rep(att_r2), b1r=rep(bias1),
                b2r=rep(bias2), dum1=d1, dum2=d2)


# ----------------------------------------------------------------------------
# Device kernel builder
# ----------------------------------------------------------------------------

def build_bass(cfg, plan, GTOT, NT, LT, debug_tables=False):
    _import_concourse()
    import concourse.bass as bass
    import concourse.bacc as bacc
    import concourse.mybir as mybir
    import concourse.tile as tile
    from concourse.masks import make_identity

    dt = mybir.dt
    Alu = mybir.AluOpType
    Act = mybir.ActivationFunctionType
    AP = bass.AP

    H, C1, NCLS = cfg["H"], cfg["C1"], cfg["NCLS"]
    NC, NEG = cfg["NCORES"], cfg["NEG_SLOPE"]
    F = cfg["F_IN"]
    KCH = F // 128
    HC1, HC2 = H * C1, H * NCLS
    REC1, REC2 = HC1 + 2 * H, HC2 + 2 * H
    P1, P2 = REC1, REC2           # table row pitches (exact records)
    NPC = NT * 128
    plan = np.asarray(plan)
    Dt = plan
    TILE_OFS = np.concatenate([[0], np.cumsum(128 * plan)])

    nc = bacc.Bacc(None)

    # ---- I/O ----
    xt = nc.declare_dram_parameter("xt", [NT, KCH, 128, 128], dt.float32, False)
    gidx = nc.declare_dram_parameter("gidx", [GTOT], dt.int32, False)
    w1 = nc.declare_dram_parameter("w1", [KCH, 128, HC1], dt.float32, False)
    w2e = nc.declare_dram_parameter("w2e", [HC1 + 1, HC2], dt.float32, False)
    consts_io = {}
    for nm, w in (("al1r", HC1), ("ar1r", HC1), ("al2r", HC2), ("ar2r", HC2),
                  ("b1r", HC1), ("b2r", NCLS)):
        consts_io[nm] = nc.declare_dram_parameter(nm, [128, w], dt.float32, False)
    dum1 = nc.declare_dram_parameter("dum1", [128, P1], dt.float16, False)
    dum2 = nc.declare_dram_parameter("dum2", [128, P2], dt.float16, False)
    outp = nc.declare_dram_parameter("out", [NPC, NCLS], dt.float32, True)
    dbg1 = dbg2 = dbg3 = dbg4 = None
    if debug_tables:
        dbg1 = nc.declare_dram_parameter("dbg1", [LT, P1], dt.float16, True)
        dbg2 = nc.declare_dram_parameter("dbg2", [LT, P2], dt.float16, True)
        dbg3 = nc.declare_dram_parameter("dbg3", [NPC, HC1 + H], dt.float32,
                                         True)
        dbg4 = nc.declare_dram_parameter("dbg4", [NPC, HC1], dt.float32, True)
        dbg5 = nc.declare_dram_parameter("dbg5", [128, int(Dt[0]) * P1],
                                         dt.float16, True)

    # ---- internal DRAM ----
    t1l = nc.dram_tensor("t1l", [LT, P1], dt.float16)
    t2l = nc.dram_tensor("t2l", [LT, P2], dt.float16)
    t1f = nc.dram_tensor("t1f", [NC * LT, P1], dt.float16,
                         addr_space="Shared")
    t2f = nc.dram_tensor("t2f", [NC * LT, P2], dt.float16,
                         addr_space="Shared")

    def bmid(ap2d, d):
        """[128, F] -> [128, d, F] broadcast via middle step-0 axis."""
        a = [list(p) for p in ap2d.ap]
        return AP(ap2d.tensor, ap2d.offset, [a[0], [0, d], a[1]])

    def bfree(apcol, n):
        """[128, 1] -> [128, n] broadcast (inner step 0)."""
        a = [list(p) for p in apcol.ap]
        return AP(apcol.tensor, apcol.offset, [a[0], [0, n]])

    def strided(ap2d, step, count):
        """[128, *] AP -> [128, count] with given element step from offset."""
        a = [list(p) for p in ap2d.ap]
        return AP(ap2d.tensor, ap2d.offset, [a[0], [step, count]])

    from contextlib import ExitStack
    with tile.TileContext(nc) as tc, ExitStack() as est:
        cpool = est.enter_context(tc.tile_pool(name="consts", bufs=1))
        xpool = est.enter_context(tc.tile_pool(name="xt", bufs=3))
        ppool = est.enter_context(tc.tile_pool(name="psum", bufs=2,
                                               space="PSUM"))
        p2pool = est.enter_context(tc.tile_pool(name="psum2", bufs=2,
                                                space="PSUM"))
        rpool = est.enter_context(tc.tile_pool(name="recs", bufs=3))
        jpool = est.enter_context(tc.tile_pool(name="xj", bufs=2))
        ipool = est.enter_context(tc.tile_pool(name="idx", bufs=3))
        dpool = est.enter_context(tc.tile_pool(name="xi", bufs=3))
        mpool = est.enter_context(tc.tile_pool(name="msg", bufs=2))
        spool = est.enter_context(tc.tile_pool(name="small", bufs=4))
        opool = est.enter_context(tc.tile_pool(name="outs", bufs=3))

        # ---- constants to SBUF ----
        w1s = cpool.tile([128, KCH, HC1], dt.float32)
        nc.sync.dma_start(out=w1s[:], in_=w1.rearrange("c p n -> p c n"))
        w2s = cpool.tile([HC1 + 1, HC2], dt.float32)
        nc.sync.dma_start(out=w2s[:], in_=w2e[:])
        csb = {}
        for nm, w in (("al1r", HC1), ("ar1r", HC1), ("al2r", HC2),
                      ("ar2r", HC2), ("b1r", HC1), ("b2r", NCLS)):
            s = cpool.tile([128, w], dt.float32, tag=nm)
            nc.sync.dma_start(out=s[:], in_=consts_io[nm][:])
            csb[nm] = s
        ident = cpool.tile([128, 128], dt.float32)
        make_identity(nc, ident[:])

        def alr_from_rec(rec, w, C, alr_const, dst_col):
            """rec[:, dst_col + h] = sum_c rec[:, h*C + c] * att[h, c]."""
            tmp = spool.tile([128, HC2], dt.float16, tag="altmp")
            nc.vector.tensor_tensor(out=tmp[:, 0:w], in0=rec[:, 0:w],
                                    in1=alr_const[:], op=Alu.mult)
            t3 = tmp[:, 0:w].rearrange("p (h c) -> p h c", c=C)
            cw = C
            while cw > 1:
                a = cw // 2
                nc.vector.tensor_tensor(out=t3[:, :, 0:a], in0=t3[:, :, 0:a],
                                        in1=t3[:, :, a:2 * a], op=Alu.add)
                if cw % 2:
                    nc.vector.tensor_tensor(
                        out=t3[:, :, 0:1], in0=t3[:, :, 0:1],
                        in1=t3[:, :, 2 * a:2 * a + 1], op=Alu.add)
                cw = a
            nc.vector.tensor_copy(out=rec[:, dst_col:dst_col + H],
                                  in_=strided(tmp[:], C, H))

        # ================= phase H1: h1 = x @ W1, write rec1 table ==========
        for t in range(NT):
            xtile = xpool.tile([128, KCH, 128], dt.float32, tag="xt")
            nc.sync.dma_start(out=xtile[:],
                              in_=xt[t].rearrange("c p n -> p c n"))
            h1p = ppool.tile([128, HC1], dt.float32, tag="h1p")
            for kc in range(KCH):
                nc.tensor.matmul(h1p[:], lhsT=xtile[:, kc, :],
                                 rhs=w1s[:, kc, :],
                                 start=(kc == 0), stop=(kc == KCH - 1))
            rec1 = rpool.tile([128, P1], dt.float16, tag="rec1")
            nc.scalar.activation(rec1[:, 0:HC1], h1p[:], Act.Copy)
            alr_from_rec(rec1, HC1, C1, csb["al1r"], HC1)
            alr_from_rec(rec1, HC1, C1, csb["ar1r"], HC1 + H)
            nc.sync.dma_start(out=t1l[t * 128:(t + 1) * 128, :], in_=rec1[:])

        nc.gpsimd.dma_start(out=t1l[NPC:NPC + 128, :], in_=dum1[:])

        tc.strict_bb_all_engine_barrier()
        nc.gpsimd.collective_compute(
            "AllGather", Alu.bypass, replica_groups=[list(range(NC))],
            ins=[t1l[:]], outs=[t1f[:]])
        tc.strict_bb_all_engine_barrier()

        # ================= edge phase (shared for both layers) ==============
        def edge_tile(layer, t, D):
            if layer == 1:
                PITCH, HC, C, table, local = P1, HC1, C1, t1f, t1l
            else:
                PITCH, HC, C, table, local = P2, HC2, NCLS, t2f, t2l
            r0 = t * 128

            idx = ipool.tile([128, D], dt.int32, tag="idx")
            gofs = int(TILE_OFS[t])
            nc.sync.dma_start(
                out=idx[:],
                in_=gidx[gofs:gofs + 128 * D].rearrange("(p d) -> p d", d=D))
            xj = jpool.tile([128, D, PITCH], dt.float16, tag="xj")
            for d in range(D):
                nc.gpsimd.indirect_dma_start(
                    out=xj[:, d, :], out_offset=None, in_=table[:],
                    in_offset=bass.IndirectOffsetOnAxis(
                        ap=idx[:, d:d + 1], axis=0))
            xi = dpool.tile([128, PITCH], dt.float16, tag="xi")
            nc.sync.dma_start(out=xi[:], in_=local[r0:r0 + 128, :])
            if debug_tables and layer == 1 and t == 0:
                nc.gpsimd.dma_start(
                    out=dbg5[:], in_=xj[:].rearrange("p a b -> p (a b)"))

            xj3 = xj[:]
            # prod[p, d, hc] = xj_h * xi_h
            prod = mpool.tile([128, D * HC], dt.float16, tag="prod")
            nc.vector.tensor_tensor(
                out=prod[:].rearrange("p (d f) -> p d f", f=HC),
                in0=xj3[:, :, 0:HC], in1=bmid(xi[:, 0:HC], D), op=Alu.mult)
            # logits tree over c (within head); result lands at c=0 cols
            p3 = prod[:].rearrange("p (dh c) -> p dh c", c=C)
            cw = C
            while cw > 1:
                a = cw // 2
                nc.vector.tensor_tensor(out=p3[:, :, 0:a], in0=p3[:, :, 0:a],
                                        in1=p3[:, :, a:2 * a], op=Alu.add)
                if cw % 2:
                    nc.vector.tensor_tensor(
                        out=p3[:, :, 0:1], in0=p3[:, :, 0:1],
                        in1=p3[:, :, 2 * a:2 * a + 1], op=Alu.add)
                cw = a
            logits = strided(prod[:], C, D * H)
            sg = spool.tile([128, Dmax * H], dt.float16, tag="sg")
            nc.scalar.activation(sg[:, 0:D * H], logits, Act.Sigmoid)
            # alpha = leaky((al_j + ar_i) * sig)
            alp = spool.tile([128, Dmax * H], dt.float16, tag="alp")
            nc.vector.tensor_tensor(
                out=alp[:, 0:D * H].rearrange("p (d h) -> p d h", h=H),
                in0=xj3[:, :, HC:HC + H],
                in1=bmid(xi[:, HC + H:HC + 2 * H], D), op=Alu.add)
            nc.vector.tensor_tensor(out=alp[:, 0:D * H], in0=alp[:, 0:D * H],
                                    in1=sg[:, 0:D * H], op=Alu.mult)
            nc.vector.scalar_tensor_tensor(
                out=alp[:, 0:D * H], in0=alp[:, 0:D * H], scalar=float(NEG),
                in1=alp[:, 0:D * H], op0=Alu.mult, op1=Alu.max)
            ex = spool.tile([128, Dmax * H], dt.float16, tag="ex")
            nc.scalar.activation(ex[:, 0:D * H], alp[:, 0:D * H], Act.Exp)

            # msg = xj_h * ex  (bcast ex over c)
            msg = mpool.tile([128, D * HC], dt.float16, tag="msg")
            exb = AP(ex[:].tensor, ex[:].offset,
                     [list(ex[:].ap[0]), [H, D], [1, H], [0, C]])
            nc.vector.tensor_tensor(
                out=msg[:].rearrange("p (d h c) -> p d h c", h=H, c=C),
                in0=xj3[:, :, 0:HC].rearrange("p d (h c) -> p d h c", c=C),
                in1=exb, op=Alu.mult)

            def dtree(flat, w, stride, out_f32):
                """halve over leading d groups of `stride`; fp32 final."""
                cw = w
                while cw > 2:
                    a = cw // 2
                    nc.vector.tensor_tensor(
                        out=flat[:, 0:a * stride], in0=flat[:, 0:a * stride],
                        in1=flat[:, a * stride:2 * a * stride], op=Alu.add)
                    if cw % 2:
                        nc.vector.tensor_tensor(
                            out=flat[:, 0:stride], in0=flat[:, 0:stride],
                            in1=flat[:, 2 * a * stride:(2 * a + 1) * stride],
                            op=Alu.add)
                    cw = a
                if cw == 2:
                    nc.vector.tensor_tensor(
                        out=out_f32, in0=flat[:, 0:stride],
                        in1=flat[:, stride:2 * stride], op=Alu.add)
                else:
                    nc.vector.tensor_copy(out=out_f32, in_=flat[:, 0:stride])

            num = opool.tile([128, HC2], dt.float32, tag="num")
            den = spool.tile([128, H], dt.float32, tag="den")
            dtree(msg, D, HC, num[:, 0:HC])
            dtree(ex, D, H, den[:])

            rd = spool.tile([128, H], dt.float32, tag="rd")
            nc.vector.reciprocal(out=rd[:], in_=den[:])
            o = opool.tile([128, HC2], dt.float32, tag="o")
            rdb = AP(rd[:].tensor, rd[:].offset,
                     [list(rd[:].ap[0]), [1, H], [0, C]])
            nc.vector.tensor_tensor(
                out=o[:, 0:HC].rearrange("p (h c) -> p h c", c=C),
                in0=num[:, 0:HC].rearrange("p (h c) -> p h c", c=C),
                in1=rdb, op=Alu.mult)

            if layer == 1:
                if debug_tables:
                    dbgt = opool.tile([128, HC1 + H], dt.float32, tag="dbgt")
                    nc.vector.tensor_copy(out=dbgt[:, 0:HC], in_=num[:, 0:HC])
                    nc.vector.tensor_copy(out=dbgt[:, HC:HC + H], in_=den[:])
                    nc.sync.dma_start(out=dbg3[r0:r0 + 128, :], in_=dbgt[:])
                # x1 = elu(o + b1); p = x1 + 1 = relu(v) + exp(min(v,0))
                v = opool.tile([128, HC1], dt.float32, tag="v")
                nc.vector.tensor_tensor(out=v[:], in0=o[:, 0:HC1],
                                        in1=csb["b1r"][:], op=Alu.add)
                mn = opool.tile([128, HC1], dt.float32, tag="mn")
                nc.vector.tensor_scalar(mn[:], v[:], 0.0, None, Alu.min)
                nc.scalar.activation(mn[:], mn[:], Act.Exp)
                pp = opool.tile([128, HC1], dt.float32, tag="pp")
                nc.vector.scalar_tensor_tensor(
                    out=pp[:], in0=v[:], scalar=0.0, in1=mn[:],
                    op0=Alu.max, op1=Alu.add)
                if debug_tables:
                    nc.sync.dma_start(out=dbg4[r0:r0 + 128, :], in_=pp[:])
                # h2 = (p | 1) @ w2e
                ptp = p2pool.tile([HC1, 128], dt.float32, tag="ptp")
                nc.tensor.transpose(ptp[:], pp[:], ident[:])
                lhs = rpool.tile([HC1 + 1, 128], dt.float32, tag="lhs")
                nc.scalar.activation(lhs[0:HC1, :], ptp[:], Act.Copy)
                nc.vector.memset(lhs[HC1:HC1 + 1, :], 1.0)
                h2p = p2pool.tile([128, HC2], dt.float32, tag="h2p")
                nc.tensor.matmul(h2p[:], lhsT=lhs[:], rhs=w2s[:],
                                 start=True, stop=True)
                rec2 = rpool.tile([128, P2], dt.float16, tag="rec2")
                nc.scalar.activation(rec2[:, 0:HC2], h2p[:], Act.Copy)
                alr_from_rec(rec2, HC2, NCLS, csb["al2r"], HC2)
                alr_from_rec(rec2, HC2, NCLS, csb["ar2r"], HC2 + H)
                nc.sync.dma_start(out=t2l[r0:r0 + 128, :], in_=rec2[:])
            else:
                # mean over heads -> +bias2 -> log_softmax
                cw = H
                o3 = o[:].rearrange("p (h c) -> p h c", c=NCLS)
                while cw > 1:
                    a = cw // 2
                    nc.vector.tensor_tensor(
                        out=o3[:, 0:a, :], in0=o3[:, 0:a, :],
                        in1=o3[:, a:2 * a, :], op=Alu.add)
                    cw = a
                z = opool.tile([128, NCLS], dt.float32, tag="z")
                nc.vector.scalar_tensor_tensor(
                    out=z[:], in0=o[:, 0:NCLS], scalar=1.0 / H,
                    in1=csb["b2r"][:], op0=Alu.mult, op1=Alu.add)
                nmx = spool.tile([128, 1], dt.float32, tag="nmx")
                nc.vector.tensor_reduce(out=nmx[:], in_=z[:],
                                        axis=mybir.AxisListType.X,
                                        op=Alu.max, negate=True)
                et = opool.tile([128, NCLS], dt.float32, tag="et")
                ssum = spool.tile([128, 1], dt.float32, tag="ssum")
                nc.scalar.activation(et[:], z[:], Act.Exp, bias=nmx[:, 0:1],
                                     accum_out=ssum[:])
                lns = spool.tile([128, 1], dt.float32, tag="lns")
                nc.scalar.activation(lns[:], ssum[:], Act.Ln)
                lp = opool.tile([128, NCLS], dt.float32, tag="lp")
                nc.vector.scalar_tensor_tensor(
                    out=lp[:], in0=z[:], scalar=nmx[:, 0:1],
                    in1=bfree(lns[:], NCLS), op0=Alu.add, op1=Alu.subtract)
                nc.sync.dma_start(out=outp[r0:r0 + 128, :], in_=lp[:])

        Dmax = int(max(Dt))
        for t in range(NT):
            edge_tile(1, t, int(Dt[t]))

        nc.gpsimd.dma_start(out=t2l[NPC:NPC + 128, :], in_=dum2[:])

        tc.strict_bb_all_engine_barrier()
        nc.gpsimd.collective_compute(
            "AllGather", Alu.bypass, replica_groups=[list(range(NC))],
            ins=[t2l[:]], outs=[t2f[:]])
        tc.strict_bb_all_engine_barrier()

        for t in range(NT):
            edge_tile(2, t, int(Dt[t]))

        if debug_tables:
            nc.gpsimd.dma_start(out=dbg1[:], in_=t1l[:])
            nc.gpsimd.dma_start(out=dbg2[:], in_=t2l[:])

    if not nc.is_finalized():
        nc.finalize()
    return nc


# ----------------------------------------------------------------------------
# Entry point
# ----------------------------------------------------------------------------

def kernel(x, edge_index, W1, att_l1, att_r1, bias1, W2, att_l2, att_r2,
           bias2, _cfg=None, _trace=False):
    cfg = dict(FULL_CFG)
    if _cfg:
        cfg.update(_cfg)
    _import_concourse()
    from concourse import bass_utils

    x = np.asarray(x, np.float32)
    edge_index = np.asarray(edge_index, np.int32)

    gidx, xt, info = _prepare(cfg, x, edge_index)
    consts = _const_inputs(cfg, W1, att_l1, att_r1, bias1, W2, att_l2,
                           att_r2, bias2)
    NC = cfg["NCORES"]

    nc = build_bass(cfg, info["plan"], info["GTOT"], info["NT"], info["LT"])

    in_maps = []
    for c in range(NC):
        m = dict(consts)
        m["xt"] = xt[c]
        m["gidx"] = gidx[c]
        in_maps.append(m)

    res = bass_utils.run_bass_kernel_spmd(
        nc, in_maps, core_ids=list(range(NC)), trace=_trace)

    N, NCLS = cfg["N"], cfg["NCLS"]
    logp = np.zeros((N, NCLS), np.float32)
    for c in range(NC):
        out_c = np.asarray(res.results[c]["out"])
        na = info["nodes_at"][c]
        real = na >= 0
        logp[na[real]] = out_c[real]
    att_loss = np.float32(0.0)
    if _trace:
        return (logp, att_loss), res
    return (logp, att_loss)
